# revision 41
# baseline (speedup 1.0000x reference)
"""Bahdanau attention kernel for Trainium2 (8 NeuronCores, data-parallel over batch).

Computes, for each batch row b:
    energy  = tanh(enc[b] @ W_e.T + (h[b] @ W_h.T) + b_attn)   # [S, DEC]
    scores  = energy @ v                                        # [S]
    out[b]  = softmax(scores)

Shapes (hardcoded): B=32, S=4096, ENC=512, DEC=512. 8 cores, 4 batch rows/core.

Device-side design (per core) -- full-fp8 matmul + 2048-wide tanh:
  - encoder outputs are host-pre-tiled [b, pg, p, k, s] with ALL FOUR
    contraction chunks cast fp8e4m3: each (c, 512-col) projection piece is
    TWO fp8 DoubleRow matmuls (2 contraction rows/cycle) -- ~2x less PE
    time than bf16. (Optional per-tile bf16 fallback for k=2,3 exists
    behind HYB for accuracy headroom.)
  - PSUM is organized as two 4-bank slots [128, 4, 512] f32 (the whole 8
    banks).  A projection group = one slot = one (b, c) over a 2048-col
    s-pair-group, so ONE tanh instruction covers 2048 columns -- halving
    the per-instruction ACT overhead.  tanh takes the host-folded
    per-(b,c) bias (dec_proj + b_attn, shipped f32) per-partition.
    Scalar-engine busy time is the bottleneck: never add ACT work.
  - v-dot: v replicated 32 wide; all four batches' scores land in one
    slot quarter at tile_position (0, 32b) (concurrent PE column groups).
    Each 2048-col boundary splits its scores across TWO ring slots: the
    framework lumps a tile's matmul writers into one dependency, so
    per-half tiles let each 1024-wide exp (accum_out carries row-sums)
    start as soon as its own half's chains land instead of after all 64
    v-dot matmuls, and the second half's matmuls don't WAR-stall on the
    first exp's read.
  - softmax tail: reduce+reciprocal, then bf16 tensor_scalar_mul quarters
    pipelined with partition-strided output DMAs alternating the qSP and
    Pool DGE rings (on the qAct ring they'd serialize behind ACT drains);
    host casts to f32.
  - weights/bias ride the qAct HWDGE ring (c=0 slice first) while enc
    tiles stream on qSP in 256KB quarters; the act-table load is forced
    first on the scalar queue (emitted later it stalls the DMA fabric
    mid-ramp); N=15 dummy matmuls on zeroed SBUF warm the PE clock during
    the head DMA wait (the enc stream is HBM-contention-limited until
    ~11us -- all 8 cores burst together, and DMA-completion semaphores
    post lazily -- so shorter warm runs just stall the PE cold).
  - this walrus build allows one sync wait per instruction; the dataflow
    is engineered for single-wait instructions and a post-pass splits any
    leftovers into wait-only drains.
"""

import os
import sys

import numpy as np

try:
    import concourse.bass as bass  # noqa: F401
except ImportError:  # toolchain lives in the trn_rl repo
    for p in ("/opt/trn_rl_repo", "/root/.axon_site/_ro/trn_rl_repo"):
        if os.path.isdir(p) and p not in sys.path:
            sys.path.insert(0, p)
    import concourse.bass as bass  # noqa: F401

import ml_dtypes

B, S, ENC, DEC = 32, 4096, 512, 512
N_CORES = 8
BPC = B // N_CORES          # batch rows per core
SG = 512                    # s-columns per matmul / PSUM bank
QPG = 4                     # 512-col quarters per pair-group
SGP = QPG * SG              # 2048 s-cols per pair-group (tanh width)
N_PG = S // SGP             # 2 pair-groups
KC = ENC // 128             # 4 contraction chunks
DC = DEC // 128             # 4 output-dim chunks
VW = 32                     # v replication width (one PE column group / batch)

# (b, pg) tiles listed here use bf16 for contraction chunks k=2,3 (accuracy
# headroom knob; empty = full fp8 everywhere)
HYB: tuple = ()

_BF16 = ml_dtypes.bfloat16
_F8 = ml_dtypes.float8_e4m3

_nc_cache = None
last_results = None         # BassKernelResults of the most recent run (for test.py)

N_WARM = 19                 # dummy PE warmup matmuls during the head DMA wait


def _build_bass():
    import concourse.tile as tile
    from concourse import mybir

    f32 = mybir.dt.float32
    bf16 = mybir.dt.bfloat16
    f8 = mybir.dt.float8e4
    Act = mybir.ActivationFunctionType

    nc = bass.Bass()

    enc8_d = nc.declare_dram_parameter(
        "enc8", [BPC, N_PG, 128, KC, SGP], f8, isOutput=False
    )
    pk8_d = nc.declare_dram_parameter("pk8", [128, KC, DEC], f8, isOutput=False)
    pkv_d = nc.declare_dram_parameter("pkv", [128, DC, VW], bf16, isOutput=False)
    bias_d = nc.declare_dram_parameter("biasd", [128, DC * BPC], f32, isOutput=False)
    if HYB:
        enc16_d = nc.declare_dram_parameter(
            "enc16", [len(HYB), 128, 2, SGP], bf16, isOutput=False
        )
        pk16_d = nc.declare_dram_parameter("pk16", [128, 2, DEC], bf16, isOutput=False)
    out_d = nc.declare_dram_parameter("out", [BPC, S], bf16, isOutput=True)

    with tile.TileContext(nc) as tc:
        # a single SBUF pool (per-tag bufs) + the PSUM pool: each pool's
        # release barrier emits per-engine exit waits, and with five pools
        # the teardown wait-chain after the last output DMA cost ~3us of
        # the measured exec window
        with (
            tc.tile_pool(name="sb", bufs=1) as sb,
            tc.tile_pool(name="psp", bufs=2, space="PSUM") as psp,
        ):
            pk8 = sb.tile([128, KC, DEC], f8)
            pkv = sb.tile([128, DC, VW], bf16)
            bias_act = sb.tile([128, DC * BPC], f32)
            warm_sb = sb.tile([128, SG], bf16)
            nc.vector.memset(warm_sb[:, :], 0.0)
            # the act_warm's implicit ACT_TABLE_LOAD must be the FIRST thing
            # on the scalar queue: emitted later it lands behind the weight
            # transfers (~10.7us), stalls the DMA fabric for 1.3us right as
            # the enc stream ramps, and delays the first real tanh.  Reading
            # warm_sb keeps it off the DMA-completion dependency chain.
            act_warm = sb.tile([128, 1], f32)
            nc.scalar.activation(act_warm[:, :], warm_sb[:, 0:1], func=Act.Tanh)
            # weights ride the qAct HWDGE ring so enc tiles on qSP don't queue
            # behind them; the first projection group only gates on the c=0
            # weight slice + bias, so those go first
            nc.scalar.dma_start(out=pk8[:, :, 0:128], in_=pk8_d[:, :, 0:128])
            nc.scalar.dma_start(out=bias_act[:, :], in_=bias_d[:, :])
            # the c=1..3 weight slices and v ride the idle Pool ring: on the
            # qAct ring their ~1.4us of transfers sit between act_warm and
            # the first tanh on the scalar queue
            nc.gpsimd.dma_start(out=pk8[:, :, 128:DEC], in_=pk8_d[:, :, 128:DEC])
            nc.gpsimd.dma_start(out=pkv[:, :, :], in_=pkv_d[:, :, :])
            if HYB:
                pk16 = sb.tile([128, 2, DEC], bf16)
                nc.scalar.dma_start(out=pk16[:, :, :], in_=pk16_d[:, :, :])

            # PE clock warmup: dummy matmuls on zeroed SBUF while DMAs land
            wp = psp.tile([128, QPG, SG], f32, tag="slot", name="warm", bufs=2)
            for _ in range(N_WARM):
                nc.tensor.matmul(
                    wp[:, 0, 0:256], warm_sb[:, 0:128], warm_sb[:, 0:256],
                    start=True, stop=True,
                )

            expd = sb.tile([128, S], bf16)
            sums8 = sb.tile([128, 2 * N_PG], f32)
            sums = sb.tile([128, 1], f32)
            recip = sb.tile([128, 1], f32)
            out_sb = sb.tile([128, S], bf16)

            def emit_vdots(pg, ven_tiles, split):
                # packed v-dots: all four batches into ONE slot quarter at
                # partitions 32*b (distinct PE column groups run concurrently);
                # exp runs 1024-wide with accum_out carrying the row-sums.
                # The framework lumps a tile's matmul writers into one dep, so
                # at the interior boundary the two halves go to SEPARATE ring
                # slots -- each exp then starts as soon as its own half's
                # chains land instead of after all 64 matmuls -- and at the
                # final boundary the second half's matmuls don't WAR-stall on
                # the first exp's read of a shared tile.
                vd = psp.tile([128, QPG, SG], f32, tag="slot", name="vd", bufs=2)
                for hf in range(2):
                    if split and hf == 1:
                        vd = psp.tile([128, QPG, SG], f32, tag="slot", name="vd2", bufs=2)
                    for q in (2 * hf, 2 * hf + 1):
                        qd = q - 2 * hf if split else q
                        for c in range(DC):
                            for b in range(BPC):
                                nc.tensor.matmul(
                                    vd[32 * b : 32 * b + 32, qd, :],
                                    pkv[:, c, :],
                                    ven_tiles[b][:, c, q * SG : (q + 1) * SG],
                                    start=(c == 0),
                                    stop=(c == DC - 1),
                                    tile_position=(0, 32 * b),
                                    skip_group_check=True,
                                )
                    sg_idx = 2 * pg + hf
                    qd0 = 0 if split else 2 * hf
                    nc.scalar.activation(
                        out=expd[:, sg_idx * (SGP // 2) : (sg_idx + 1) * (SGP // 2)],
                        in_=vd[:, qd0 : qd0 + 2, :],
                        func=Act.Exp,
                        accum_out=sums8[:, sg_idx : sg_idx + 1],
                    )

            prev_vd = None
            for pg in range(N_PG):
                en_tiles = []
                for b in range(BPC):
                    hyb = (b, pg) in HYB
                    e8t = sb.tile([128, KC, SGP], f8, tag="e8", name="e8", bufs=6)
                    for q in range(QPG):
                        nc.sync.dma_start(
                            out=e8t[:, :, q * SG : (q + 1) * SG],
                            in_=enc8_d[b, pg, :, :, q * SG : (q + 1) * SG],
                        )
                    if hyb:
                        hix = HYB.index((b, pg))
                        e16t = sb.tile([128, 2, SGP], bf16, tag="e16", name="e16", bufs=2)
                        for q in range(QPG):
                            nc.sync.dma_start(
                                out=e16t[:, :, q * SG : (q + 1) * SG],
                                in_=enc16_d[hix, :, :, q * SG : (q + 1) * SG],
                            )
                    en2 = sb.tile([128, DC, SGP], bf16, tag="en2", name="en2", bufs=8)
                    en_tiles.append(en2)
                    for c in range(DC):
                        # previous pair-group's v-dots slot in after this
                        # group's first projection group: by then its last tanh
                        # is done, so the PE reaches them wait-free
                        if b == 0 and c == 1 and prev_vd is not None:
                            emit_vdots(*prev_vd, split=True)
                            prev_vd = None
                        pp = psp.tile([128, QPG, SG], f32, tag="slot", name="pp", bufs=2)
                        for q in range(QPG):
                            # k=0,1 fused in one fp8 DoubleRow matmul
                            nc.tensor.matmul(
                                pp[:, q, :],
                                pk8[:, 0:2, c * 128 : (c + 1) * 128],
                                e8t[:, 0:2, q * SG : (q + 1) * SG],
                                start=True,
                                stop=False,
                                perf_mode=mybir.MatmulPerfMode.DoubleRow,
                            )
                            if hyb:
                                for k in range(2):
                                    nc.tensor.matmul(
                                        pp[:, q, :],
                                        pk16[:, k, c * 128 : (c + 1) * 128],
                                        e16t[:, k, q * SG : (q + 1) * SG],
                                        start=False,
                                        stop=(k == 1),
                                    )
                            else:
                                nc.tensor.matmul(
                                    pp[:, q, :],
                                    pk8[:, 2:4, c * 128 : (c + 1) * 128],
                                    e8t[:, 2:4, q * SG : (q + 1) * SG],
                                    start=False,
                                    stop=True,
                                    perf_mode=mybir.MatmulPerfMode.DoubleRow,
                                )
                        nc.scalar.activation(
                            out=en2[:, c, :],
                            in_=pp[:, :, :],
                            func=Act.Tanh,
                            bias=bias_act[:, c * BPC + b : c * BPC + b + 1],
                        )
                prev_vd = (pg, en_tiles)
            emit_vdots(*prev_vd, split=True)

            # softmax tail: one reciprocal, then mul quarters pipelined with
            # partition-strided output DMAs on the idle DVE/Pool HWDGE rings
            # (on the ACT/SP rings they'd serialize behind engine drains)
            nc.vector.reduce_sum(sums[:, :], sums8[:, :], axis=mybir.AxisListType.X)
            nc.vector.reciprocal(recip[:, :], sums[:, :])
            for q in range(4):
                eng = (nc.sync, nc.gpsimd, nc.gpsimd, nc.sync)[q]
                nc.vector.tensor_scalar_mul(
                    out=out_sb[:, q * SG * 2 : (q + 1) * SG * 2],
                    in0=expd[:, q * SG * 2 : (q + 1) * SG * 2],
                    scalar1=recip[:, :],
                )
                eng.dma_start(
                    out=out_d[0:BPC, q * SG * 2 : (q + 1) * SG * 2],
                    in_=out_sb[0 : 32 * BPC : 32, q * SG * 2 : (q + 1) * SG * 2],
                )

    _split_multi_waits(nc)
    return nc


def _split_multi_waits(nc):
    """This walrus build allows ONE sync wait per instruction. The kernel body
    is engineered to respect that; Tile's auto-emitted tail drain is not (it
    waits on every processor). Split any multi-wait instruction into a chain
    of single-wait drains on the same engine followed by the original."""
    from concourse import mybir

    for bb in nc.main_func.blocks:
        new_insts = []
        for ins in bb.instructions:
            si = getattr(ins, "sync_info", None)
            if si is not None and si.on_wait and len(si.on_wait) > 1:
                waits = list(si.on_wait)
                for w in waits[:-1]:
                    d = mybir.InstNoOp(
                        name=nc.get_next_instruction_name(),
                        ins=[],
                        outs=[],
                    )
                    d.engine = ins.engine
                    d.sync_info = mybir.SyncInfo(on_wait=[w], on_update=[])
                    nc.register_instruction(d)
                    new_insts.append(d)
                si.on_wait = waits[-1:]
            new_insts.append(ins)
        bb.instructions[:] = new_insts


def _get_nc():
    global _nc_cache
    if _nc_cache is None:
        _nc_cache = _build_bass()
    return _nc_cache


def _prep_in_maps(decoder_hidden, encoder_outputs, W_attn, b_attn, v):
    decoder_hidden = np.asarray(decoder_hidden, dtype=np.float32)
    encoder_outputs = np.asarray(encoder_outputs, dtype=np.float32)
    W_attn = np.asarray(W_attn, dtype=np.float32)
    b_attn = np.asarray(b_attn, dtype=np.float32)
    v = np.asarray(v, dtype=np.float32)

    W_h = W_attn[:, :DEC]           # [d_out, d_in]
    W_e = W_attn[:, DEC:]           # [d_out, e]

    W_eT = W_e.T.reshape(KC, 128, DEC).transpose(1, 0, 2)  # [128, KC, DEC] f32
    pk8 = np.ascontiguousarray(W_eT).astype(_F8)
    pk16 = np.ascontiguousarray(W_eT[:, 2:4]).astype(_BF16)
    pkv = np.ascontiguousarray(
        np.broadcast_to(v.astype(_BF16).reshape(DC, 128).T[:, :, None], (128, DC, VW))
    )

    # host-folded per-row bias: dec_proj + b_attn  [B, DEC]
    bias_full = decoder_hidden @ W_h.T + b_attn

    # [B, S, E] -> [B, N_PG, 128(p=e%128), KC(e//128), SGP(s)], all chunks fp8
    enc_t = encoder_outputs.reshape(B, N_PG, SGP, KC, 128).transpose(0, 1, 4, 3, 2)
    enc8 = np.ascontiguousarray(enc_t).astype(_F8)

    in_maps = []
    for core in range(N_CORES):
        sl = slice(core * BPC, (core + 1) * BPC)
        # [BPC, DC, 128] -> biasd[:, c*BPC + b] = bias_full[b, c, :]
        biasd = np.ascontiguousarray(
            bias_full[sl].reshape(BPC, DC, 128).transpose(2, 1, 0).reshape(128, DC * BPC)
        )
        m = {"enc8": enc8[sl], "pk8": pk8, "pkv": pkv, "biasd": biasd}
        if HYB:
            m["pk16"] = pk16
            m["enc16"] = np.stack(
                [
                    np.ascontiguousarray(
                        enc_t[core * BPC + b, pg, :, 2:4, :]
                    ).astype(_BF16)
                    for (b, pg) in HYB
                ]
            )
        in_maps.append(m)
    return in_maps


def _ensure_ntff_hook():
    """The agent image's ``antenv`` lacks ``axon_hooks``; synthesize it with a
    ctypes-based NTFF profile hook against the injected libaxon (trace runs only)."""
    try:
        from antenv.axon_hooks import get_axon_ntff_profile_hook  # noqa: F401

        return
    except ImportError:
        pass

    import contextlib
    import ctypes
    import types

    so_path = "/opt/axon/libaxon_pjrt.so"
    hook = None
    if os.path.exists(so_path):
        lib = ctypes.CDLL(so_path)
        if hasattr(lib, "axon_start_nrt_profile"):
            lib.axon_start_nrt_profile.argtypes = [
                ctypes.POINTER(ctypes.c_int64),
                ctypes.c_size_t,
            ]
            lib.axon_start_nrt_profile.restype = ctypes.c_int64
            lib.axon_stop_nrt_profile.argtypes = [ctypes.c_char_p]
            lib.axon_stop_nrt_profile.restype = ctypes.c_int64

            @contextlib.contextmanager
            def _hook(output_dir, device_ids):
                import jax

                jax.devices()
                if device_ids:
                    ids = (ctypes.c_int64 * len(device_ids))(*device_ids)
                    rc = lib.axon_start_nrt_profile(ids, len(device_ids))
                else:
                    rc = lib.axon_start_nrt_profile(None, 0)
                if rc != 0:
                    raise RuntimeError(f"axon_start_nrt_profile rc={rc}")
                try:
                    yield
                finally:
                    n = lib.axon_stop_nrt_profile(str(output_dir).encode())
                    if n <= 0:
                        print(f"ntff capture wrote {n} files", file=sys.stderr)

            hook = _hook

    holder = {"h": hook}
    mod = types.ModuleType("antenv.axon_hooks")
    mod.get_axon_ntff_profile_hook = lambda: holder["h"]
    mod.set_axon_ntff_profile_hook = lambda h: holder.__setitem__("h", h)
    sys.modules["antenv.axon_hooks"] = mod
    import antenv

    antenv.axon_hooks = mod


def kernel(decoder_hidden, encoder_outputs, W_attn, b_attn, v):
    global last_results
    import concourse.bass_utils as bass_utils
    from concourse.bass_utils import run_bass_kernel_spmd

    nc = _get_nc()
    in_maps = _prep_in_maps(decoder_hidden, encoder_outputs, W_attn, b_attn, v)

    trace = os.environ.get("BAHDANAU_TRACE", "0") == "1"
    kwargs = {}
    if trace:
        _ensure_ntff_hook()
        bass_utils.upload_artifacts = lambda tmpdir: str(tmpdir)  # no bucket here
        kwargs["trace"] = True
        tmpdir = os.environ.get("BAHDANAU_TRACE_DIR")
        if tmpdir:
            import uuid

            tmpdir = os.path.join(tmpdir, uuid.uuid4().hex[:8])
            os.makedirs(tmpdir, exist_ok=True)
            kwargs["tmpdir"] = tmpdir

    res = run_bass_kernel_spmd(nc, in_maps, core_ids=list(range(N_CORES)), **kwargs)
    last_results = res
    out = np.concatenate([res.results[c]["out"] for c in range(N_CORES)], axis=0)
    return out.astype(np.float32)
